# revision 4
# baseline (speedup 1.0000x reference)
"""Trainium2 Bass kernel for a 3D attention block.

Reference computation (per batch b):
    xf = x[b].reshape(C, N)                       # C=256, N=4096
    q  = Wq @ xf + bq                             # [32, N]
    k  = Wk @ xf + bk                             # [32, N]
    v  = Wv @ xf + bv                             # [256, N]
    P  = softmax(q.T @ k, axis=-1)                # [N(m), N(n)]
    out[c, m] = sum_n v[c, n] * P[m, n]
    result = gamma * out + x[b]

Sharding: 8 cores = 2 batches x 4 chunks of 1024 query rows (m).
SPMD trick: every core receives x pre-rolled along n by -1024*j so its
query chunk sits at columns 0:1024.  Softmax rowsum and PV are
permutation-invariant in n, so k/v simply use the rolled order and no
per-core program differences are needed.

On-device layout (per core) is transpose-free:
    S^T[n, m] = k^T q   (n on partitions)  -> exp on ACT -> P^T in SBUF
    out[c, m] = sum over n-tiles of vT[n-tile, c].T @ P^T[n-tile, m]
Softmax max-subtraction is skipped (|S| <= ~25, exp stays in fp32/bf16
range).

Design notes (PSUM is the binding constraint: 8 banks of [128,512]f32):
  - acc-mh0 (PV accumulator for m-cols 0:512):        2 banks
  - S^T tiles [128,1024], double buffered:            4 banks
  - projection psums (cycled) -> acc-mh1 at sweep 2:  2 banks
  All 32 P^T tiles are kept in SBUF (64KB/partition), so PV for m-cols
  512:1024 runs as a second pure-PE sweep after the S^T/exp loop; the
  mh0 epilogue (rowsum -> 1/x -> broadcast -> scale+residual) overlaps
  that sweep.  Rowsum is chain-accumulated on DVE in bf16 (4 chains +
  tree) and finished by a single ones^T matmul pair - no per-tile
  rowsum matmuls on PE.
  - v projection is one fp8e4 DoubleRow matmul per n-tile (contracts
    both 128-halves of C at once).  Host-validated absmax ~6e-2 on an
    output scale of 5.3 (rel ~1.1e-2 < 2e-2 gate); q/k stay fp16 (fp8
    there blows the softmax: absmax 0.35).

ATTN_KERNEL_REPEATS=<R> emits the body R times in one NEFF (timing via
slope; outputs are idempotent). ATTN_KERNEL_TRACE=1 captures an NTFF
profile via run_bass_kernel_spmd(trace=True).
ATTN_V_FP8=0 falls back to an fp16 v projection (2 matmuls/tile).
"""

import os

import numpy as np

import concourse.bass as bass
import concourse.mybir as mybir
import concourse.tile as tile
from concourse import bacc
from concourse.bass_utils import run_bass_kernel_spmd

F32 = mybir.dt.float32
F16 = mybir.dt.float16
BF16 = mybir.dt.bfloat16
F8 = mybir.dt.float8e4

C = 256
C8 = 32
N = 4096  # 16*16*16 voxels
MCHUNK = 1024  # query rows per core
NT = N // 128  # 32 key tiles
NCORES = 8
V_FP8 = int(os.environ.get("ATTN_V_FP8", "1"))

# info stashed by the last kernel() call (for test harnesses)
LAST_RESULTS = None


def _emit_body(nc, tc, io, rep):
    xf16, x8, wqk, wv8, bqk, bv, gamma, out = io
    r = f"_{rep}"
    with (
        tc.tile_pool(name="big" + r, bufs=1) as big,
        tc.tile_pool(name="epi" + r, bufs=2) as epi,
        tc.tile_pool(name="pacc" + r, bufs=1, space="PSUM") as pacc,
        tc.tile_pool(name="pst" + r, bufs=2, space="PSUM") as pst,
        tc.tile_pool(name="proj" + r, bufs=2, space="PSUM") as prj,
    ):
        def chunk_sl(ch):
            return slice(ch * 512, (ch + 1) * 512)

        # ---- input DMAs.  wqk + the first two x chunks feed q/k(0);
        # everything else streams behind them.  Small constants go on
        # the gpsimd queue so they don't delay the x stream.
        wqk_t = big.tile([128, 2, 2 * C8], F16, name="wqk_t" + r)
        nc.sync.dma_start(wqk_t[:], wqk[:])
        xf_t = big.tile([128, 2, N], F16, name="xf_t" + r)
        for ch in range(2):
            nc.sync.dma_start(xf_t[:, :, chunk_sl(ch)], xf16[:, :, chunk_sl(ch)])
        if V_FP8:
            wv8_t = big.tile([128, 2, C], F8, name="wv8_t" + r)
            nc.sync.dma_start(wv8_t[:], wv8[:])
            x8_t = big.tile([128, 2, N], F8, name="x8_t" + r)
            nc.sync.dma_start(x8_t[:, :, 0:1024], x8[:, :, 0:1024])
        else:
            wv_t = big.tile([128, 2, C], F16, name="wv_t" + r)
            nc.sync.dma_start(wv_t[:], wv8[:])
        for ch in range(2, 8):
            nc.sync.dma_start(xf_t[:, :, chunk_sl(ch)], xf16[:, :, chunk_sl(ch)])
            if V_FP8 and ch % 2 == 1:
                sl2 = slice((ch - 1) * 512, (ch + 1) * 512)
                nc.sync.dma_start(x8_t[:, :, sl2], x8[:, :, sl2])

        bqk_t = big.tile([C8, 2], F32, name="bqk_t" + r)
        nc.gpsimd.dma_start(bqk_t[:], bqk[:])
        bv_b = big.tile([128, C], F32, name="bv_b" + r)
        nc.gpsimd.dma_start(
            bv_b[:], bass.AP(tensor=bv, offset=0, ap=[[0, 128], [1, C]])
        )
        gamma_b = big.tile([128, 1], F32, name="gamma_b" + r)
        nc.gpsimd.dma_start(
            gamma_b[:], bass.AP(tensor=gamma, offset=0, ap=[[0, 128], [1, 1]])
        )

        bq_t = bqk_t[:, 0:1]
        bk_t = bqk_t[:, 1:2]
        ones_t = big.tile([128, 1], BF16, name="ones_t" + r)
        nc.vector.memset(ones_t[:], 1.0)
        ones_row = big.tile([1, 128], F32, name="ones_row" + r)
        nc.vector.memset(ones_row[:], 1.0)

        q_sb = big.tile([C8, MCHUNK], F16, name="q_sb" + r)
        k_sb = big.tile([C8, N], F16, name="k_sb" + r)
        vt_sb = big.tile([128, NT, C], BF16, name="vt_sb" + r)
        pt_all = big.tile([128, NT, MCHUNK], BF16, name="pt_all" + r)
        pac = [big.tile([128, MCHUNK], BF16, name=f"pac{j}" + r) for j in range(4)]

        # ---- projection emitters (psums cycle through the prj pool) --
        def emit_q():
            for mh in range(2):
                sl = chunk_sl(mh)
                q_ps = prj.tile([128, 512], F32, tag="pj", name=f"q_ps{mh}" + r)
                nc.tensor.matmul(
                    q_ps[:C8, :], wqk_t[:, 0, 0:C8], xf_t[:, 0, sl],
                    start=True, stop=False,
                )
                nc.tensor.matmul(
                    q_ps[:C8, :], wqk_t[:, 1, 0:C8], xf_t[:, 1, sl],
                    start=False, stop=True,
                )
                nc.vector.tensor_scalar_add(q_sb[:, sl], q_ps[:C8, :], bq_t)

        def emit_k(ch):
            sl = chunk_sl(ch)
            k_ps = prj.tile([128, 512], F32, tag="pj", name=f"k_ps{ch}" + r)
            nc.tensor.matmul(
                k_ps[:C8, :], wqk_t[:, 0, C8 : 2 * C8], xf_t[:, 0, sl],
                start=True, stop=False,
            )
            nc.tensor.matmul(
                k_ps[:C8, :], wqk_t[:, 1, C8 : 2 * C8], xf_t[:, 1, sl],
                start=False, stop=True,
            )
            nc.vector.tensor_scalar_add(k_sb[:, sl], k_ps[:C8, :], bk_t)

        def emit_vt(nt):
            sl = slice(nt * 128, (nt + 1) * 128)
            v_ps = prj.tile([128, 512], F32, tag="pj", name=f"v_ps{nt}" + r)
            if V_FP8:
                nc.tensor.matmul(
                    v_ps[:, :C], x8_t[:, :, sl], wv8_t[:],
                    start=True, stop=True,
                    perf_mode=mybir.MatmulPerfMode.DoubleRow,
                )
            else:
                nc.tensor.matmul(
                    v_ps[:, :C], xf_t[:, 0, sl], wv_t[:, 0, :],
                    start=True, stop=False,
                )
                nc.tensor.matmul(
                    v_ps[:, :C], xf_t[:, 1, sl], wv_t[:, 1, :],
                    start=False, stop=True,
                )
            nc.vector.tensor_add(vt_sb[:, nt, :], v_ps[:, :C], bv_b[:])

        # ---- main loop: S^T + exp + DVE P-accum + PV for m-half 0 ----
        acc0 = [
            pacc.tile([128, 512], F32, name=f"acc0h{h}" + r) for h in range(2)
        ]

        def emit_st_exp(nt):
            ksl = k_sb[:, nt * 128 : (nt + 1) * 128]
            st = pst.tile([128, MCHUNK], F32, tag="st", name=f"st{nt}" + r)
            for mh in range(2):
                msl = slice(mh * 512, (mh + 1) * 512)
                nc.tensor.matmul(
                    st[:, msl], ksl, q_sb[:, msl], start=True, stop=True
                )
            nc.scalar.activation(
                pt_all[:, nt, :], st[:], mybir.ActivationFunctionType.Exp
            )
            if nt < 4:
                nc.vector.tensor_copy(pac[nt], pt_all[:, nt, :])
            else:
                nc.vector.tensor_add(pac[nt % 4], pac[nt % 4], pt_all[:, nt, :])

        def emit_pv0(nt):
            for h in range(2):
                nc.tensor.matmul(
                    acc0[h][:],
                    vt_sb[:, nt, h * 128 : (h + 1) * 128],
                    pt_all[:, nt, 0:512],
                    start=nt == 0, stop=nt == NT - 1,
                )

        LAG = 2
        emit_q()
        emit_k(0)
        for nt in range(4):
            emit_vt(nt)
        for i in range(8):
            if i + 1 < 8:
                emit_k(i + 1)
                for nt in range(4 * (i + 1), 4 * (i + 1) + 4):
                    emit_vt(nt)
            for nt in range(4 * i, 4 * i + 4):
                emit_st_exp(nt)
                if nt >= LAG:
                    emit_pv0(nt - LAG)
        for nt in range(NT - LAG, NT):
            emit_pv0(nt)

        # rowsum tree on DVE: pac0 += pac1, pac2 += pac3, pac0 += pac2
        nc.vector.tensor_add(pac[0], pac[0], pac[1])
        nc.vector.tensor_add(pac[2], pac[2], pac[3])
        nc.vector.tensor_add(pac[0], pac[0], pac[2])

        # ---- sweep 2 (PV m-half 1) with the mh0 epilogue overlapped --
        acc1 = [
            prj.tile([128, 512], F32, tag="pj", name=f"acc1h{h}" + r)
            for h in range(2)
        ]

        def emit_pv1(nt):
            for h in range(2):
                nc.tensor.matmul(
                    acc1[h][:],
                    vt_sb[:, nt, h * 128 : (h + 1) * 128],
                    pt_all[:, nt, 512:1024],
                    start=nt == 0, stop=nt == NT - 1,
                )

        rs_tile = pst.tile([128, MCHUNK], F32, tag="st", name="rs_t" + r)
        rs_ps = rs_tile[:1, :]
        gr_tile = pst.tile([128, MCHUNK], F32, tag="st", name="gr_ps" + r)
        rinv = epi.tile([1, MCHUNK], F32, name="rinv" + r)
        grecip_b = big.tile([128, MCHUNK], F32, name="gr_b" + r)
        res = [
            epi.tile([128, MCHUNK], F32, tag=f"res{h}", name=f"res{h}" + r)
            for h in range(2)
        ]
        accs = {0: acc0, 1: acc1}

        def emit_epilogue(mh):
            msl = slice(mh * 512, (mh + 1) * 512)
            nc.vector.reciprocal_approx_fast(rinv[:, msl], rs_ps[:, msl])
            nc.tensor.matmul(
                gr_tile[:, msl], ones_row[:], rinv[:, msl], start=True, stop=True
            )
            nc.scalar.activation(
                grecip_b[:, msl], gr_tile[:, msl],
                mybir.ActivationFunctionType.Copy, scale=gamma_b[:],
            )
            for h in range(2):
                # GPSIMD cannot read PSUM: mul (PSUM src) stays on DVE,
                # the all-SBUF residual add for h=1 goes to GPSIMD.
                nc.vector.tensor_mul(
                    res[h][:, msl], accs[mh][h][:], grecip_b[:, msl]
                )
                eng = nc.vector if h == 0 else nc.gpsimd
                eng.tensor_add(res[h][:, msl], res[h][:, msl], xf_t[:, h, msl])
                nc.sync.dma_start(out[:, h, msl], res[h][:, msl])

        for nt in range(0, 12):
            emit_pv1(nt)
        for mh in range(2):
            msl = slice(mh * 512, (mh + 1) * 512)
            nc.tensor.matmul(
                rs_ps[:, msl], ones_t[:], pac[0][:, msl], start=True, stop=True
            )
        for nt in range(12, 20):
            emit_pv1(nt)
        emit_epilogue(0)
        for nt in range(20, NT):
            emit_pv1(nt)
        emit_epilogue(1)


def _build(repeats=1):
    nc = bacc.Bacc("TRN2", target_bir_lowering=False, debug=False, num_devices=NCORES)

    xf16 = nc.dram_tensor("xf16", [128, 2, N], F16, kind="ExternalInput")
    x8 = nc.dram_tensor("x8", [128, 2, N], F8 if V_FP8 else F16, kind="ExternalInput")
    wqk = nc.dram_tensor("wqk", [128, 2, 2 * C8], F16, kind="ExternalInput")
    wv8 = nc.dram_tensor(
        "wv8", [128, 2, C], F8 if V_FP8 else F16, kind="ExternalInput"
    )
    bqk = nc.dram_tensor("bqk", [C8, 2], F32, kind="ExternalInput")
    bv = nc.dram_tensor("bv", [1, C], F32, kind="ExternalInput")
    gamma = nc.dram_tensor("gamma", [1, 1], F32, kind="ExternalInput")
    out = nc.dram_tensor("out", [128, 2, MCHUNK], F32, kind="ExternalOutput")
    io = (xf16, x8, wqk, wv8, bqk, bv, gamma, out)

    with tile.TileContext(nc) as tc:
        for rep in range(repeats):
            _emit_body(nc, tc, io, rep)

    nc.compile()
    return nc


_NC_CACHE = {}


def _get_nc(repeats=1):
    if repeats not in _NC_CACHE:
        _NC_CACHE[repeats] = _build(repeats)
    return _NC_CACHE[repeats]


def _in_maps(x, Wq, bq, Wk, bk, Wv, bv, gamma):
    import ml_dtypes

    f8dt = ml_dtypes.float8_e4m3 if V_FP8 else np.float16
    xflat = x.reshape(2, C, N)
    # [C, N] -> [128, 2, N] with c = h*128 + p
    xh16 = xflat.astype(np.float16).reshape(2, 2, 128, N).transpose(0, 2, 1, 3)
    xh8 = xflat.astype(f8dt).reshape(2, 2, 128, N).transpose(0, 2, 1, 3)
    # [128, 2, 64]: wqk[p, h, o] = [Wq.T | Wk.T][h*128+p, o]
    wqk_full = np.concatenate([Wq.T, Wk.T], axis=1).astype(np.float16)  # [C, 64]
    wqk2 = np.ascontiguousarray(wqk_full.reshape(2, 128, 2 * C8).transpose(1, 0, 2))
    # [128, 2, 256]: wv8[p, h, co] = Wv[co, h*128+p]
    wv82 = np.ascontiguousarray(
        Wv.T.astype(f8dt).reshape(2, 128, C).transpose(1, 0, 2)
    )
    bqk2 = np.ascontiguousarray(
        np.stack([bq.reshape(C8), bk.reshape(C8)], axis=1).astype(np.float32)
    )
    bv2 = np.ascontiguousarray(bv.reshape(1, C))
    g2 = np.ascontiguousarray(gamma.reshape(1, 1))

    maps = []
    for core in range(NCORES):
        b, j = core // 4, core % 4
        roll = -j * MCHUNK
        maps.append(
            {
                "xf16": np.ascontiguousarray(np.roll(xh16[b], roll, axis=2)),
                "x8": np.ascontiguousarray(np.roll(xh8[b], roll, axis=2)),
                "wqk": wqk2,
                "wv8": wv82,
                "bqk": bqk2,
                "bv": bv2,
                "gamma": g2,
            }
        )
    return maps


def kernel(x, Wq, bq, Wk, bk, Wv, bv, gamma):
    global LAST_RESULTS
    x = np.ascontiguousarray(np.asarray(x, dtype=np.float32))
    args = [np.asarray(a, dtype=np.float32) for a in (Wq, bq, Wk, bk, Wv, bv, gamma)]

    B, Cc, D, H, W = x.shape
    assert (B, Cc, D * H * W) == (2, C, N), x.shape

    repeats = int(os.environ.get("ATTN_KERNEL_REPEATS", "1"))
    nc = _get_nc(repeats)
    maps = _in_maps(x, *args)
    kwargs = {}
    if int(os.environ.get("ATTN_KERNEL_TRACE", "0")):
        kwargs = dict(
            trace=True,
            trace_cores=[0],
            tmpdir=os.environ.get("ATTN_KERNEL_TRACE_DIR"),
        )
    res = run_bass_kernel_spmd(nc, maps, core_ids=list(range(NCORES)), **kwargs)
    LAST_RESULTS = res

    outf = np.empty((B, C, N), dtype=np.float32)
    for core in range(NCORES):
        b, j = core // 4, core % 4
        o = res.results[core]["out"]  # [128, 2, 1024]
        outf[b][:, j * MCHUNK : (j + 1) * MCHUNK] = o.transpose(1, 0, 2).reshape(
            C, MCHUNK
        )
    return outf.reshape(B, Cc, D, H, W)
